# revision 11
# baseline (speedup 1.0000x reference)
"""Equivariant conv layer (GNN message passing) on 8 TRN2 NeuronCores.

Math (per reference), reassociated to cut FLOPs ~12x:
  h_es = MLP(edge_attr)                                  [E, 64]
  msg[e, :160] = [h_es[e] | rel_pos[e]] @ CR[src[e]]     (unified bilinear)
  aggr = scatter_mean(msg, dst)                          [N, 160]
  out0e = silu(aggr0e) + feat0e ; gates = silu(aggr0e @ Wg + bg)
  out1o = aggr1o * gates + feat1o ; out = [out0e | out1o.flat]

CR[m] is a per-node [68, 160] matrix:
  rows 0:64 (h):  cols 0:64     T[m][h,o]   = sum_p feat0e[m,p] w_ss[p,h,o]
                  cols 64+c*3+x P[m][h,c,x] = sum_d feat1o[m,d,x] w_vs[d,h,c]
  rows 64:67 (r): Q rows (w_vv / w_sv / w_cross terms; host-precomputed,
                  ~0.4 GFLOP of the ~55 GFLOP total)

Distribution: edges sorted by src; 8 contiguous src-node ranges with ~E/8
edges each. Per core: edge-MLP + CR precompute (TensorE) + per-src-group
matmuls (stationary = [h_es|rel_pos] group block, moving = CR[node]) ->
per-edge messages -> dst-sorted dma_gather + indicator-matmul aggregation
into a full-N partial -> ReduceScatter -> gate/residual tail on the core's
N/8 node slice. Host does index bookkeeping only.
"""
import numpy as np
import ml_dtypes

import concourse.bacc as bacc
import concourse.bass as bass
import concourse.tile as tile
import concourse.mybir as mybir
from concourse.bass_utils import run_bass_kernel_spmd
from concourse.masks import make_identity

# problem dims
N, E, D0, D1, H = 20000, 640000, 64, 32, 64
NC = 8
F = 160            # msg width = 64 + 96
K = 68             # stationary rows: 64 h + 3 relpos + 1 pad
S = 64             # edge slots per subgroup
WIN = 128          # subgroups per window
TILE_N = 128       # nodes per aggregation tile
MSG_C = 256        # msg row bf16 cols (512B, %256B for gather)
NTILE_PAD = 160    # node tiles (20480 padded node rows)
NPAD = NTILE_PAD * TILE_N          # 20480
NSEG_OUT = NPAD // NC              # 2560 output rows per core
TG = 8             # tiles per gather group
NTG = NTILE_PAD // TG              # 20 gather groups
BKT = 128          # slots per (tile, seg) bucket
SEG = 16384        # msg-table rows per gather segment (< 2^15)

bf16 = mybir.dt.bfloat16
f32 = mybir.dt.float32
i16 = mybir.dt.int16


def _wrap_idx(idx):
    """int16 gather index layout: [128, n/16], idx j at (j%16, j//16), x8."""
    n = idx.shape[0]
    assert n % 16 == 0
    return np.tile(idx.reshape(n // 16, 16).T, (8, 1)).astype(np.int16)


def _build_host(inputs):
    feat0e = np.asarray(inputs["feat0e"], np.float32)
    feat1o = np.asarray(inputs["feat1o"], np.float32)
    edge_attr = np.asarray(inputs["edge_attr"], np.float32)
    pos = np.asarray(inputs["pos"], np.float32)
    ei = np.asarray(inputs["edge_index"])
    W = {k: np.asarray(inputs[k], np.float32) for k in
         ("W1", "b1", "W2", "b2", "W3", "b3", "w_ss", "w_vv", "w_sv", "w_vs",
          "w_cross", "Wg", "bg")}
    src = ei[0].astype(np.int64)
    dst = ei[1].astype(np.int64)
    relpos = pos[dst] - pos[src]                       # [E, 3]

    # ---- per-node Q rows (host: ~0.4 GFLOP index-light prep) ----
    Q0 = np.einsum("ndx,do->nxo", feat1o, W["w_vv"])   # [N,3,64]
    q1 = feat0e @ W["w_sv"]                            # [N,32]
    M1 = np.einsum("dc,nda->nac", W["w_cross"], feat1o)  # [N,3,32]
    eps = np.zeros((3, 3, 3), np.float32)
    eps[0, 1, 2] = eps[1, 2, 0] = eps[2, 0, 1] = 1
    eps[0, 2, 1] = eps[2, 1, 0] = eps[1, 0, 2] = -1
    Q2 = np.einsum("xab,nac->nbcx", eps, M1)           # [N,b,c,x]
    Q12 = Q2.copy()
    for x in range(3):
        Q12[:, x, :, x] += q1                          # Q1: delta_{x,b} q1[c]
    Qrows = np.concatenate([Q0, Q12.reshape(N, 3, 96)], axis=2)  # [N,3,160]

    # ---- src-sorted core ranges (balance edge counts, node-aligned) ----
    cnt_src = np.bincount(src, minlength=N)
    cum = np.concatenate([[0], np.cumsum(cnt_src)])
    bounds = [0]
    for c in range(1, NC):
        bounds.append(int(np.searchsorted(cum, c * E / NC)))
    bounds.append(N)
    order = np.argsort(src, kind="stable")

    Gs = []
    for c in range(NC):
        ns = cnt_src[bounds[c]:bounds[c + 1]]
        Gs.append(int(np.maximum((ns + S - 1) // S, 1).sum()))
    G = ((max(Gs) + WIN - 1) // WIN) * WIN
    ESL = G * S
    NWIN = G // WIN
    NSEGS = (ESL + SEG - 1) // SEG

    cnt_dst = np.bincount(dst, minlength=N)
    inv_cnt_full = (1.0 / np.maximum(cnt_dst, 1)).astype(np.float32)

    def bd(w):
        z = np.zeros((128, 128), np.float32)
        z[:64, :64] = w
        z[64:, 64:] = w
        return z.astype(ml_dtypes.bfloat16)

    # cols o*64+h of w_ss[p,h,o]
    w_ss_perm = np.ascontiguousarray(
        W["w_ss"].transpose(0, 2, 1).reshape(64, 64 * 64)).astype(ml_dtypes.bfloat16)
    # cols c*64+h of w_vs[d,h,c]
    w_vs_perm = np.ascontiguousarray(
        W["w_vs"].transpose(0, 2, 1).reshape(32, 32 * 64)).astype(ml_dtypes.bfloat16)

    shared = {
        "w1": bd(W["W1"]), "w2": bd(W["W2"]), "w3": bd(W["W3"]),
        "b1": np.tile(W["b1"], 2).reshape(128, 1).astype(np.float32),
        "b2": np.tile(W["b2"], 2).reshape(128, 1).astype(np.float32),
        "b3": np.tile(W["b3"], 2).reshape(128, 1).astype(np.float32),
        "wssp": w_ss_perm, "wvsp": w_vs_perm,
        "wg": W["Wg"].astype(np.float32),
        "bg": W["bg"].reshape(32, 1).astype(np.float32),
    }

    cores = []
    for c in range(NC):
        m0, m1 = bounds[c], bounds[c + 1]
        eids = order[cum[m0]:cum[m1]]
        ns = cnt_src[m0:m1]
        nsub = np.maximum((ns + S - 1) // S, 1)
        g_of_node = np.concatenate([[0], np.cumsum(nsub)])
        g_used = int(g_of_node[-1])
        assert g_used <= G
        sub_node = np.zeros(G, np.int64)
        slot_eid = np.full(ESL, -1, np.int64)
        estart = np.concatenate([[0], np.cumsum(ns)])
        for i in range(m1 - m0):
            g0 = g_of_node[i]
            k = int(ns[i])
            sub_node[g0:g_of_node[i + 1]] = m0 + i
            for j in range(int(nsub[i])):
                off = j * S
                take = max(min(S, k - off), 0)
                if take > 0:
                    sl = (g0 + j) * S
                    e0 = estart[i] + off
                    slot_eid[sl:sl + take] = eids[e0:e0 + take]
        real = slot_eid >= 0
        r_eid = slot_eid[real]
        r_slot = np.nonzero(real)[0]

        attr2 = np.zeros((128, ESL // 2), np.float32)
        ea = edge_attr[r_eid]
        even = r_slot % 2 == 0
        attr2[:64, r_slot[even] // 2] = ea[even].T
        attr2[64:, r_slot[~even] // 2] = ea[~even].T
        relp = np.zeros((4, ESL), np.float32)
        relp[:3, r_slot] = relpos[r_eid].T

        f0T = np.zeros((64, G), np.float32)
        f0T[:, :g_used] = feat0e[sub_node[:g_used]].T
        f1T = np.zeros((3 * 32, G), np.float32)
        f1T[:, :g_used] = feat1o[sub_node[:g_used]].transpose(2, 1, 0).reshape(96, -1)
        # Q rows, feature-major per window: [3, NWIN, F, WIN]
        qh = np.zeros((3, NWIN, F, WIN), np.float32)
        qv = Qrows[sub_node[:g_used]].transpose(1, 2, 0)   # [3, F, g_used]
        for w in range(NWIN):
            lo, hi = w * WIN, min((w + 1) * WIN, g_used)
            if hi > lo:
                qh[:, w, :, :hi - lo] = qv[:, :, lo:hi]

        # ---- aggregation buckets ----
        d_e = dst[r_eid]
        t_e = d_e // TILE_N
        s_e = r_slot // SEG
        idx16 = (r_slot % SEG).astype(np.int16)
        dloc = (d_e % TILE_N).astype(np.int64)
        idx_arr = np.zeros((NTILE_PAD, NSEGS, BKT), np.int16)
        ind_arr = np.zeros((NTILE_PAD, NSEGS, BKT, TILE_N), np.float32)
        bucket_of = t_e * NSEGS + s_e
        bord = np.argsort(bucket_of, kind="stable")
        ub, counts = np.unique(bucket_of[bord], return_counts=True)
        assert counts.max() <= BKT, f"bucket overflow {counts.max()}"
        off = 0
        for b, k in zip(ub, counts):
            t, sgm = divmod(int(b), NSEGS)
            sel = bord[off:off + k]
            off += k
            idx_arr[t, sgm, :k] = idx16[sel]
            idx_arr[t, sgm, k:] = idx16[sel[0]]        # duplicate pad (IND=0)
            ind_arr[t, sgm, np.arange(k), dloc[sel]] = 1.0
        idx_wrapped = np.zeros((NTG, NSEGS, 128, TG * BKT // 16), np.int16)
        for tg in range(NTG):
            for sgm in range(NSEGS):
                flat = idx_arr[tg * TG:(tg + 1) * TG, sgm].reshape(-1)
                idx_wrapped[tg, sgm] = _wrap_idx(flat)

        rows = np.arange(c * NSEG_OUT, (c + 1) * NSEG_OUT)
        ok = rows < N
        invc = np.zeros((NSEG_OUT, 1), np.float32)
        invc[ok, 0] = inv_cnt_full[rows[ok]]
        f0res = np.zeros((NSEG_OUT, 64), np.float32)
        f0res[ok] = feat0e[rows[ok]]
        f1res = np.zeros((NSEG_OUT, 96), np.float32)
        f1res[ok] = feat1o[rows[ok]].reshape(-1, 96)

        cores.append({
            "attr2": attr2.astype(ml_dtypes.bfloat16),
            "relp": relp.astype(ml_dtypes.bfloat16),
            "f0T": f0T.astype(ml_dtypes.bfloat16),
            "f1T": f1T.astype(ml_dtypes.bfloat16),
            "qh": np.ascontiguousarray(qh.reshape(3, NWIN * F * WIN)
                                       ).astype(ml_dtypes.bfloat16),
            "aggidx": idx_wrapped,
            "ind": np.ascontiguousarray(
                ind_arr.reshape(NTG, TG, NSEGS, BKT, TILE_N)
                .transpose(0, 3, 2, 1, 4)
                .reshape(NTG, BKT, NSEGS * TG * TILE_N)).astype(ml_dtypes.bfloat16),
            "invc": invc, "f0res": f0res, "f1res": f1res,
        })
    dims = dict(G=G, ESL=ESL, NWIN=NWIN, NSEGS=NSEGS)
    return shared, cores, dims


def _build_program(dims):
    G, ESL, NWIN, NSEGS = (dims[k] for k in ("G", "ESL", "NWIN", "NSEGS"))
    nc = bacc.Bacc("TRN2", target_bir_lowering=False, debug=False, num_devices=NC)

    attr2 = nc.dram_tensor("attr2", [128, ESL // 2], bf16, kind="ExternalInput")
    relp = nc.dram_tensor("relp", [4, ESL], bf16, kind="ExternalInput")
    f0T = nc.dram_tensor("f0T", [64, G], bf16, kind="ExternalInput")
    f1T = nc.dram_tensor("f1T", [96, G], bf16, kind="ExternalInput")
    qh = nc.dram_tensor("qh", [3, NWIN * F * WIN], bf16, kind="ExternalInput")
    aggidx = nc.dram_tensor("aggidx", [NTG, NSEGS, 128, TG * BKT // 16], i16,
                            kind="ExternalInput")
    ind = nc.dram_tensor("ind", [NTG, BKT, NSEGS * TG * TILE_N], bf16,
                         kind="ExternalInput")
    invc = nc.dram_tensor("invc", [NSEG_OUT, 1], f32, kind="ExternalInput")
    f0res = nc.dram_tensor("f0res", [NSEG_OUT, 64], f32, kind="ExternalInput")
    f1res = nc.dram_tensor("f1res", [NSEG_OUT, 96], f32, kind="ExternalInput")
    w1 = nc.dram_tensor("w1", [128, 128], bf16, kind="ExternalInput")
    w2 = nc.dram_tensor("w2", [128, 128], bf16, kind="ExternalInput")
    w3 = nc.dram_tensor("w3", [128, 128], bf16, kind="ExternalInput")
    b1 = nc.dram_tensor("b1", [128, 1], f32, kind="ExternalInput")
    b2 = nc.dram_tensor("b2", [128, 1], f32, kind="ExternalInput")
    b3 = nc.dram_tensor("b3", [128, 1], f32, kind="ExternalInput")
    wssp = nc.dram_tensor("wssp", [64, 64 * 64], bf16, kind="ExternalInput")
    wvsp = nc.dram_tensor("wvsp", [32, 32 * 64], bf16, kind="ExternalInput")
    wg = nc.dram_tensor("wg", [64, 32], f32, kind="ExternalInput")
    bg = nc.dram_tensor("bg", [32, 1], f32, kind="ExternalInput")

    out = nc.dram_tensor("out", [NSEG_OUT, F], f32, kind="ExternalOutput")

    SLW = WIN * S          # slots per window (8192)
    CW = SLW // 2          # attr2 cols per window (4096)
    Silu = mybir.ActivationFunctionType.Silu
    Copy = mybir.ActivationFunctionType.Copy

    with tile.TileContext(nc) as tc:
        with (
            tc.tile_pool(name="const", bufs=1) as cpool,
            tc.tile_pool(name="dram", bufs=1, space="DRAM") as dpool,
        ):
            msg_dram = dpool.tile([ESL, MSG_C], bf16)
            partial = dpool.tile([NPAD, F], f32)
            rs_out = dpool.tile([NSEG_OUT, F], f32)

            w1t = cpool.tile([128, 128], bf16)
            w2t = cpool.tile([128, 128], bf16)
            w3t = cpool.tile([128, 128], bf16)
            b1t = cpool.tile([128, 1], f32)
            b2t = cpool.tile([128, 1], f32)
            b3t = cpool.tile([128, 1], f32)
            wsst = cpool.tile([64, 64 * 64], bf16)
            wvst = cpool.tile([32, 32 * 64], bf16)
            f0Tt = cpool.tile([64, G], bf16)
            f1Tt0 = cpool.tile([32, G], bf16)
            f1Tt1 = cpool.tile([32, G], bf16)
            f1Tt2 = cpool.tile([32, G], bf16)
            f1Tts = (f1Tt0, f1Tt1, f1Tt2)
            for t_, s_ in ((w1t, w1), (w2t, w2), (w3t, w3), (b1t, b1),
                           (b2t, b2), (b3t, b3), (wsst, wssp), (wvst, wvsp),
                           (f0Tt, f0T)):
                nc.sync.dma_start(t_[:], s_[:])
            for x in range(3):
                nc.sync.dma_start(f1Tts[x][:], f1T[x * 32:(x + 1) * 32, :])

            # ============ phase 1: per-window MLP + CR + pass-A ============
            with (
                tc.tile_pool(name="mlpin", bufs=2) as mlpin,
                tc.tile_pool(name="hbuf", bufs=2) as hbuf,
                tc.tile_pool(name="hrbuf", bufs=2) as hrbuf,
                tc.tile_pool(name="crbuf", bufs=1) as crbuf,
                tc.tile_pool(name="msgbuf", bufs=2) as msgbuf,
                tc.tile_pool(name="mp", bufs=2, space="PSUM") as mp,
                tc.tile_pool(name="crp", bufs=3, space="PSUM") as crp,
                tc.tile_pool(name="ap", bufs=2, space="PSUM") as ap,
            ):
                for w in range(NWIN):
                    sl0 = w * SLW
                    a_t = mlpin.tile([128, CW], bf16, tag="attr")
                    nc.sync.dma_start(a_t[:], attr2[:, w * CW:(w + 1) * CW])
                    hr = hrbuf.tile([K, SLW], bf16, tag="hr")
                    nc.sync.dma_start(hr[64:68, :], relp[:, sl0:sl0 + SLW])

                    h1 = hbuf.tile([128, CW], bf16, tag="h1")
                    h2 = hbuf.tile([128, CW], bf16, tag="h2")
                    for (win_, bin_, src_t, dst_t, fn) in (
                        (w1t, b1t, a_t, h1, Silu),
                        (w2t, b2t, h1, h2, Silu),
                    ):
                        for j in range(CW // 512):
                            pm = mp.tile([128, 512], f32, tag="mlp")
                            nc.tensor.matmul(pm[:], lhsT=win_[:],
                                             rhs=src_t[:, j * 512:(j + 1) * 512],
                                             start=True, stop=True)
                            nc.scalar.activation(
                                dst_t[:, j * 512:(j + 1) * 512], pm[:], fn,
                                bias=bin_[:])
                    h3 = hbuf.tile([128, CW], bf16, tag="h1")
                    for j in range(CW // 512):
                        pm = mp.tile([128, 512], f32, tag="mlp")
                        nc.tensor.matmul(pm[:], lhsT=w3t[:],
                                         rhs=h2[:, j * 512:(j + 1) * 512],
                                         start=True, stop=True)
                        nc.vector.tensor_scalar_add(
                            h3[:, j * 512:(j + 1) * 512], pm[:], b3t[:])
                    hr2 = hr[0:64, :].rearrange("p (c two) -> p c two", two=2)
                    nc.vector.tensor_copy(hr2[:, :, 0], h3[0:64, :])
                    nc.vector.tensor_copy(hr2[:, :, 1], h3[64:128, :])

                    # --- CR window (feature-major: col = f*WIN + n) ---
                    cr = crbuf.tile([K, F * WIN], bf16, tag="cr")
                    nc.sync.dma_start(
                        cr[64:67, :],
                        qh[:, w * F * WIN:(w + 1) * F * WIN])
                    f0w = f0Tt[:, w * WIN:(w + 1) * WIN]
                    for o in range(64):
                        pc = crp.tile([64, WIN], f32, tag="crx")
                        nc.tensor.matmul(pc[:], lhsT=wsst[:, o * 64:(o + 1) * 64],
                                         rhs=f0w, start=True, stop=True)
                        if o % 2 == 0:
                            nc.vector.tensor_copy(
                                cr[0:64, o * WIN:(o + 1) * WIN], pc[:])
                        else:
                            nc.scalar.copy(
                                cr[0:64, o * WIN:(o + 1) * WIN], pc[:])
                    for x in range(3):
                        f1w = f1Tts[x][:, w * WIN:(w + 1) * WIN]
                        for ci in range(32):
                            fcol = 64 + ci * 3 + x
                            pc = crp.tile([64, WIN], f32, tag="crx")
                            nc.tensor.matmul(
                                pc[:], lhsT=wvst[:, ci * 64:(ci + 1) * 64],
                                rhs=f1w, start=True, stop=True)
                            if ci % 2 == 0:
                                nc.vector.tensor_copy(
                                    cr[0:64, fcol * WIN:(fcol + 1) * WIN], pc[:])
                            else:
                                nc.scalar.copy(
                                    cr[0:64, fcol * WIN:(fcol + 1) * WIN], pc[:])

                    # --- pass A ---
                    crv = cr[:, :].rearrange("p (f n) -> p f n", n=WIN)
                    mw = msgbuf.tile([128, (WIN // 2) * F], bf16, tag="msg")
                    for gp in range(WIN // 2):
                        pa = ap.tile([128, F], f32, tag="pa")
                        for half in range(2):
                            g_loc = gp * 2 + half
                            nc.tensor.matmul(
                                pa[half * 64:(half + 1) * 64, :],
                                lhsT=hr[:, g_loc * S:(g_loc + 1) * S],
                                rhs=crv[:, :, g_loc],
                                start=True, stop=True)
                        if gp % 2 == 0:
                            nc.vector.tensor_copy(mw[:, gp * F:(gp + 1) * F], pa[:])
                        else:
                            nc.scalar.copy(mw[:, gp * F:(gp + 1) * F], pa[:])
                    nc.sync.dma_start(
                        msg_dram[sl0:sl0 + SLW, 0:F]
                        .rearrange("(c p) f -> p c f", p=128),
                        mw[:].rearrange("p (c f) -> p c f", f=F))

            # ============ phase 2: aggregation ============
            with (
                tc.tile_pool(name="gbuf", bufs=2) as gbuf,
                tc.tile_pool(name="ibuf", bufs=2) as ibuf,
                tc.tile_pool(name="pstg", bufs=2) as pstg,
                tc.tile_pool(name="idxb", bufs=2) as idxb,
                tc.tile_pool(name="gp", bufs=4, space="PSUM") as gp_,
            ):
                for tg in range(NTG):
                    stg = pstg.tile([128, TG * F], f32, tag="stg")
                    gts = []
                    for sgm in range(NSEGS):
                        ix = idxb.tile([128, TG * BKT // 16], i16, tag="ix",
                                       bufs=NSEGS + 2)
                        nc.sync.dma_start(ix[:], aggidx[tg, sgm])
                        gt = gbuf.tile([128, TG * MSG_C], bf16, tag="gt",
                                       bufs=NSEGS + 2)
                        lo = sgm * SEG
                        hi = min(lo + SEG, ESL)
                        nc.gpsimd.dma_gather(
                            gt[:].rearrange("p (c e) -> p c e", e=MSG_C),
                            msg_dram[lo:hi, :],
                            ix[:], TG * BKT, TG * BKT, MSG_C)
                        gts.append(gt)
                    it_t = ibuf.tile([128, NSEGS * TG * TILE_N], bf16, tag="ind")
                    nc.sync.dma_start(it_t[:], ind[tg])
                    for tl in range(TG):
                        pt = gp_.tile([128, F], f32, tag="agg")
                        for sgm in range(NSEGS):
                            nc.tensor.matmul(
                                pt[:],
                                lhsT=it_t[:, (sgm * TG + tl) * TILE_N:
                                          (sgm * TG + tl + 1) * TILE_N],
                                rhs=gts[sgm][:, tl * MSG_C: tl * MSG_C + F],
                                start=(sgm == 0), stop=(sgm == NSEGS - 1))
                        if tl % 2 == 0:
                            nc.vector.tensor_copy(stg[:, tl * F:(tl + 1) * F], pt[:])
                        else:
                            nc.scalar.copy(stg[:, tl * F:(tl + 1) * F], pt[:])
                    nc.sync.dma_start(
                        partial[tg * TG * TILE_N:(tg + 1) * TG * TILE_N, :]
                        .rearrange("(t p) f -> p t f", p=128),
                        stg[:].rearrange("p (t f) -> p t f", f=F))

                nc.gpsimd.collective_compute(
                    "ReduceScatter", mybir.AluOpType.add,
                    replica_groups=[list(range(NC))],
                    ins=[partial[:]], outs=[rs_out[:]])

            # ============ phase 3: tail (gate + residual) ============
            with (
                tc.tile_pool(name="tbuf", bufs=2) as tb,
                tc.tile_pool(name="tp", bufs=2, space="PSUM") as tp,
                tc.tile_pool(name="tc1", bufs=1) as tc1,
            ):
                ident = tc1.tile([128, 128], f32)
                make_identity(nc, ident)
                wgt = tc1.tile([64, 32], f32)
                nc.sync.dma_start(wgt[:], wg[:])
                bgt = tc1.tile([32, 1], f32)
                nc.sync.dma_start(bgt[:], bg[:])
                NT = NSEG_OUT // 128
                ic_t = tc1.tile([128, NT], f32)
                nc.sync.dma_start(
                    ic_t[:], invc[:].rearrange("(c p) one -> p (c one)", p=128))
                aggrT = tc1.tile([64, NSEG_OUT], f32)

                aggs = []
                for t in range(NT):
                    ag = tb.tile([128, F], f32, tag="ag", bufs=NT)
                    nc.sync.dma_start(ag[:], rs_out[t * 128:(t + 1) * 128, :])
                    nc.vector.tensor_scalar_mul(ag[:], ag[:], ic_t[:, t:t + 1])
                    pt = tp.tile([64, 128], f32, tag="tr")
                    nc.tensor.transpose(pt[:], ag[:, 0:64], ident[:])
                    nc.vector.tensor_copy(aggrT[:, t * 128:(t + 1) * 128], pt[:])
                    aggs.append(ag)
                gatesT = tc1.tile([32, NSEG_OUT], f32)
                for j in range(NSEG_OUT // 512):
                    pg = tp.tile([32, 512], f32, tag="gate")
                    nc.tensor.matmul(pg[:], lhsT=wgt[:],
                                     rhs=aggrT[:, j * 512:(j + 1) * 512],
                                     start=True, stop=True)
                    nc.scalar.activation(gatesT[:, j * 512:(j + 1) * 512],
                                         pg[:], Silu, bias=bgt[:])
                for t in range(NT):
                    ot = tb.tile([128, F], f32, tag="ot")
                    nc.scalar.activation(ot[:, 0:64], aggs[t][:, 0:64], Silu)
                    f0t_ = tb.tile([128, 64], f32, tag="f0r")
                    nc.sync.dma_start(f0t_[:], f0res[t * 128:(t + 1) * 128, :])
                    nc.vector.tensor_add(ot[:, 0:64], ot[:, 0:64], f0t_[:])
                    pgt = tp.tile([128, 32], f32, tag="trb")
                    nc.tensor.transpose(pgt[:], gatesT[:, t * 128:(t + 1) * 128],
                                        ident[0:32, 0:32])
                    gt_ = tb.tile([128, 32], f32, tag="gn")
                    nc.vector.tensor_copy(gt_[:], pgt[:])
                    f1t_ = tb.tile([128, 96], f32, tag="f1r")
                    nc.sync.dma_start(f1t_[:], f1res[t * 128:(t + 1) * 128, :])
                    nc.vector.tensor_tensor(
                        out=ot[:, 64:160].rearrange("p (c x) -> p c x", x=3),
                        in0=aggs[t][:, 64:160].rearrange("p (c x) -> p c x", x=3),
                        in1=gt_[:].to_broadcast([128, 32, 3]),
                        op=mybir.AluOpType.mult)
                    nc.vector.tensor_add(ot[:, 64:160], ot[:, 64:160], f1t_[:])
                    nc.sync.dma_start(out[t * 128:(t + 1) * 128, :], ot[:])

    nc.compile()
    return nc


_CACHE = {}


def _get_program(dims):
    key = tuple(sorted(dims.items()))
    if key not in _CACHE:
        _CACHE[key] = _build_program(dims)
    return _CACHE[key]


def _make_in_maps(shared, cores):
    in_maps = []
    for c in range(NC):
        m = dict(shared)
        m.update(cores[c])
        in_maps.append({k: np.ascontiguousarray(v) for k, v in m.items()})
    return in_maps


def kernel(**inputs) -> np.ndarray:
    shared, cores, dims = _build_host(inputs)
    nc = _get_program(dims)
    in_maps = _make_in_maps(shared, cores)
    r = run_bass_kernel_spmd(nc, in_maps, core_ids=list(range(NC)), trace=False)
    segs = [r.results[c]["out"] for c in range(NC)]
    return np.concatenate(segs, axis=0)[:N].astype(np.float32)
